# revision 8
# baseline (speedup 1.0000x reference)
"""Trainium2 Bass kernel for nn_KDHR (gnn_message_passing).

Math reduction: with S[d,s] = #edges (s->d) over N_SH=1195 nodes,
each GCN-mean layer is  h = tanh((S @ x @ W.T + cnt*b) / max(cnt,1)),
cnt = row sums of S.  So the 1M-edge message passing collapses to a
dense (1195,1195) count matrix (built once) + small dense matmuls.

Device layout: everything feature-major ("T layout", features on the
128-partition axis) so BatchNorm / bias / norms are per-partition ops.
Batch (16384) is sharded 2048 rows/core across 8 cores; BN statistics
are all-reduced (one tiny [64,2] collective).
"""

import os
import sys

for _p in ("/root/.axon_site", "/root/.axon_site/_ro/trn_rl_repo",
           "/root/.axon_site/_ro/pypackages", "/opt/trn_rl_repo", "/opt/pypackages"):
    if os.path.isdir(_p) and _p not in sys.path:
        sys.path.append(_p)

import numpy as np

import concourse.bass as bass
import concourse.mybir as mybir
import concourse.tile as tile
from concourse import bacc
from concourse.bass_utils import run_bass_kernel_spmd
from concourse.masks import make_identity

N_USER, N_ITEM, N_SH, D = 805, 390, 1195, 64
B, E, NCORES = 16384, 1048576, 8
BS = B // NCORES  # 2048 batch rows per core
BN_EPS = 1e-5
NORM_EPS = 1e-12
F32 = mybir.dt.float32

# contraction chunks over the node dim (1195 = 9*128 + 43)
KCH = [(k, min(128, N_SH - k)) for k in range(0, N_SH, 128)]
# chunks over the item dim (390 = 3*128 + 6)
CCH = [(c, min(128, N_ITEM - c)) for c in range(0, N_ITEM, 128)]


def _nsl(n, step=512):
    return [(s, min(step, n - s)) for s in range(0, n, step)]


def _build(trace=False):
    nc = bacc.Bacc("TRN2", target_bir_lowering=False, debug=False,
                   num_devices=NCORES)

    pt = nc.declare_dram_parameter("pt", [BS, N_ITEM], F32, isOutput=False).ap()
    st = nc.declare_dram_parameter("st", [N_SH, N_SH], F32, isOutput=False).ap()
    emb = nc.declare_dram_parameter("emb", [N_SH, D], F32, isOutput=False).ap()
    w1t = nc.declare_dram_parameter("w1t", [D, D], F32, isOutput=False).ap()
    w2t = nc.declare_dram_parameter("w2t", [D, D], F32, isOutput=False).ap()
    mwt = nc.declare_dram_parameter("mwt", [D, D], F32, isOutput=False).ap()
    vecs = nc.declare_dram_parameter("vecs", [D, 5], F32, isOutput=False).ap()
    out = nc.declare_dram_parameter("out", [BS, N_USER], F32, isOutput=True).ap()

    from contextlib import ExitStack
    with tile.TileContext(nc) as tc, ExitStack() as ctx:
        pools = {
            "cst": ctx.enter_context(tc.tile_pool(name="cst", bufs=1)),
            "big": ctx.enter_context(tc.tile_pool(name="big", bufs=1)),
            "sb": ctx.enter_context(tc.tile_pool(name="sb", bufs=1)),
            "ptp": ctx.enter_context(tc.tile_pool(name="ptp", bufs=2)),
            "xp": ctx.enter_context(tc.tile_pool(name="xp", bufs=1)),
            "outp": ctx.enter_context(tc.tile_pool(name="outp", bufs=2)),
            "ptr": ctx.enter_context(tc.tile_pool(name="ptr", bufs=2, space="PSUM")),
            "pbig": ctx.enter_context(tc.tile_pool(name="pbig", bufs=1, space="PSUM")),
            "pout": ctx.enter_context(tc.tile_pool(name="pout", bufs=1, space="PSUM")),
            "dram": ctx.enter_context(tc.tile_pool(name="dram", bufs=1, space="DRAM")),
        }
        _body(nc, tc, pools, pt, st, emb, w1t, w2t, mwt, vecs, out)

    nc.compile()
    return nc


def _body(nc, tc, pools, pt, st, emb, w1t, w2t, mwt, vecs, out):
    AF = mybir.ActivationFunctionType
    ALU = mybir.AluOpType
    AX = mybir.AxisListType
    cst, big, sb = pools["cst"], pools["big"], pools["sb"]
    ptp, xp, outp = pools["ptp"], pools["xp"], pools["outp"]
    ptr, pbig, dram = pools["ptr"], pools["pbig"], pools["dram"]

    # ---- constants ----
    ident = cst.tile([128, 128], F32, tag="ident")
    make_identity(nc, ident[:])
    ones_col = cst.tile([128, 1], F32, tag="ones_col")   # column of ones
    nc.vector.memset(ones_col[:], 1.0)
    ones_row = cst.tile([1, D], F32, tag="ones_row")     # 1 x 64 of ones
    nc.vector.memset(ones_row[:], 1.0)

    w1t_sb = cst.tile([D, D], F32, tag="w1t")
    nc.sync.dma_start(w1t_sb[:], w1t[:, :])
    w2t_sb = cst.tile([D, D], F32, tag="w2t")
    nc.sync.dma_start(w2t_sb[:], w2t[:, :])
    mwt_sb = cst.tile([D, D], F32, tag="mwt")
    nc.sync.dma_start(mwt_sb[:], mwt[:, :])
    vec_sb = cst.tile([D, 5], F32, tag="vecs")           # b1,b2,mb,gam,bet
    nc.sync.dma_start(vec_sb[:], vecs[:, :])
    b1 = vec_sb[:, 0:1]
    b2 = vec_sb[:, 1:2]
    mb = vec_sb[:, 2:3]
    gam = vec_sb[:, 3:4]
    bet = vec_sb[:, 4:5]

    # ---- load S^T (10 chunks of [<=128, 1195]) and emb chunks ----
    st_sb, x1_sb = [], []
    for i, (k0, kn) in enumerate(KCH):
        t = sb.tile([128, N_SH], F32, tag=f"st{i}")
        nc.sync.dma_start(t[:kn, :], st[k0:k0 + kn, :])
        st_sb.append(t)
        x = sb.tile([128, D], F32, tag=f"x1{i}")
        nc.sync.dma_start(x[:kn, :], emb[k0:k0 + kn, :])
        x1_sb.append(x)

    # ---- cnt = column sums of S^T -> recm[64,1195] = bcast 1/max(cnt,1) ----
    cnt_ps = pbig.tile([1, N_SH], F32, tag="big")
    for i, (k0, kn) in enumerate(KCH):
        for ns, nn in _nsl(N_SH):
            nc.tensor.matmul(cnt_ps[:, ns:ns + nn], ones_col[:kn, :],
                             st_sb[i][:kn, ns:ns + nn],
                             start=(i == 0), stop=(i == len(KCH) - 1))
    rec1 = sb.tile([1, N_SH], F32, tag="rec1")
    nc.vector.tensor_scalar_max(rec1[:], cnt_ps[:], 1.0)
    nc.vector.reciprocal(rec1[:], rec1[:])
    rep_ps = pbig.tile([D, N_SH], F32, tag="big")
    for ns, nn in _nsl(N_SH):
        nc.tensor.matmul(rep_ps[:, ns:ns + nn], ones_row[:, :],
                         rec1[:, ns:ns + nn], start=True, stop=True)
    recm = big.tile([D, N_SH], F32, tag="recm")
    nc.vector.tensor_copy(recm[:], rep_ps[:])

    # ---- x1T via PE transpose ----
    x1t = big.tile([D, N_SH], F32, tag="x1t")
    for i, (k0, kn) in enumerate(KCH):
        tp = ptr.tile([128, 128], F32, tag="tr")
        nc.tensor.transpose(tp[:D, :kn], x1_sb[i][:kn, :D], ident[:kn, :kn])
        nc.vector.tensor_copy(x1t[:, k0:k0 + kn], tp[:D, :kn])

    # ---- layer 1: AT = x1^T @ S^T ; h1T = tanh(AT*W1^T-ish scaled) ----
    at_ps = pbig.tile([D, N_SH], F32, tag="big")
    for i, (k0, kn) in enumerate(KCH):
        for ns, nn in _nsl(N_SH):
            nc.tensor.matmul(at_ps[:, ns:ns + nn], x1_sb[i][:kn, :D],
                             st_sb[i][:kn, ns:ns + nn],
                             start=(i == 0), stop=(i == len(KCH) - 1))
    at_sb = big.tile([D, N_SH], F32, tag="ab")
    nc.vector.tensor_copy(at_sb[:], at_ps[:])
    h1p_ps = pbig.tile([D, N_SH], F32, tag="big")
    for ns, nn in _nsl(N_SH):
        nc.tensor.matmul(h1p_ps[:, ns:ns + nn], w1t_sb[:, :],
                         at_sb[:, ns:ns + nn], start=True, stop=True)
    h1t = big.tile([D, N_SH], F32, tag="h1t")
    nc.vector.tensor_mul(h1t[:], h1p_ps[:], recm[:])
    nc.scalar.activation(h1t[:], h1t[:], AF.Tanh, bias=b1)

    # transpose h1T -> h1 natural (lhsT for layer 2)
    h1_sb = []
    for i, (k0, kn) in enumerate(KCH):
        tp = ptr.tile([128, 128], F32, tag="tr")
        nc.tensor.transpose(tp[:kn, :D], h1t[:, k0:k0 + kn], ident[:D, :D])
        h = sb.tile([128, D], F32, tag=f"h1_{i}")
        nc.vector.tensor_copy(h[:kn, :], tp[:kn, :D])
        h1_sb.append(h)

    # ---- layer 2 ----
    bt_ps = pbig.tile([D, N_SH], F32, tag="big")
    for i, (k0, kn) in enumerate(KCH):
        for ns, nn in _nsl(N_SH):
            nc.tensor.matmul(bt_ps[:, ns:ns + nn], h1_sb[i][:kn, :D],
                             st_sb[i][:kn, ns:ns + nn],
                             start=(i == 0), stop=(i == len(KCH) - 1))
    bt_sb = big.tile([D, N_SH], F32, tag="ab")
    nc.vector.tensor_copy(bt_sb[:], bt_ps[:])
    h2p_ps = pbig.tile([D, N_SH], F32, tag="big")
    for ns, nn in _nsl(N_SH):
        nc.tensor.matmul(h2p_ps[:, ns:ns + nn], w2t_sb[:, :],
                         bt_sb[:, ns:ns + nn], start=True, stop=True)
    h2t = big.tile([D, N_SH], F32, tag="h2t")
    nc.vector.tensor_mul(h2t[:], h2p_ps[:], recm[:])
    nc.scalar.activation(h2t[:], h2t[:], AF.Tanh, bias=b2)

    # ---- norms -> esT/ehT ----
    # row norms of x1 (per node) as a row vector, via ones-matmul on x1t^2
    sqx = big.tile([D, N_SH], F32, tag="sq")
    nc.scalar.activation(sqx[:], x1t[:], AF.Square)
    rn_ps = pbig.tile([1, N_SH], F32, tag="big")
    for ns, nn in _nsl(N_SH):
        nc.tensor.matmul(rn_ps[:, ns:ns + nn], ones_col[:D, :],
                         sqx[:, ns:ns + nn], start=True, stop=True)
    rn = sb.tile([1, N_SH], F32, tag="rn")
    nc.scalar.activation(rn[:], rn_ps[:], AF.Sqrt)
    nc.vector.tensor_scalar_max(rn[:], rn[:], NORM_EPS)
    nc.vector.reciprocal(rn[:], rn[:])
    repn_ps = pbig.tile([D, N_SH], F32, tag="big")
    for ns, nn in _nsl(N_SH):
        nc.tensor.matmul(repn_ps[:, ns:ns + nn], ones_row[:, :],
                         rn[:, ns:ns + nn], start=True, stop=True)
    repn = big.tile([D, N_SH], F32, tag="repn")
    nc.vector.tensor_copy(repn[:], repn_ps[:])

    # column norms of h2 (per feature), separately for user/item slices
    hsq = big.tile([D, N_SH], F32, tag="sq")
    nc.scalar.activation(hsq[:], h2t[:], AF.Square)
    rcu = sb.tile([D, 2], F32, tag="rcu")
    nc.vector.tensor_reduce(rcu[:, 0:1], hsq[:, 0:N_USER], axis=AX.X, op=ALU.add)
    nc.vector.tensor_reduce(rcu[:, 1:2], hsq[:, N_USER:N_SH], axis=AX.X, op=ALU.add)
    nc.scalar.activation(rcu[:], rcu[:], AF.Sqrt)
    nc.vector.tensor_scalar_max(rcu[:], rcu[:], NORM_EPS)
    nc.vector.reciprocal(rcu[:], rcu[:])

    # ehT[64,805] / esT[64,390]
    eht = big.tile([D, N_USER], F32, tag="eht")
    nc.vector.tensor_mul(eht[:], x1t[:, 0:N_USER], repn[:, 0:N_USER])
    tmpu = big.tile([D, N_USER], F32, tag="tmp")
    nc.vector.tensor_scalar_mul(tmpu[:], h2t[:, 0:N_USER], rcu[:, 0:1])
    nc.vector.tensor_add(eht[:], eht[:], tmpu[:])
    est = big.tile([D, N_ITEM], F32, tag="est")
    nc.vector.tensor_mul(est[:], x1t[:, N_USER:N_SH], repn[:, N_USER:N_SH])
    tmpi = big.tile([D, N_ITEM], F32, tag="tmp")
    nc.vector.tensor_scalar_mul(tmpi[:], h2t[:, N_USER:N_SH], rcu[:, 1:2])
    nc.vector.tensor_add(est[:], est[:], tmpi[:])

    # es natural [390,64] (lhsT for e_synd), via PE transpose
    es_sb = []
    for i, (c0, cn) in enumerate(CCH):
        tp = ptr.tile([128, 128], F32, tag="tr")
        nc.tensor.transpose(tp[:cn, :D], est[:, c0:c0 + cn], ident[:D, :D])
        e = sb.tile([128, D], F32, tag=f"es{i}")
        nc.vector.tensor_copy(e[:cn, :], tp[:cn, :D])
        es_sb.append(e)

    # ---- batch stage: X = P^T in SBUF via PE transposes ----
    x_sb = [xp.tile([128, BS], F32, tag=f"X{i}", name=f"X{i}")
            for i in range(len(CCH))]
    for bi in range(BS // 128):
        p = ptp.tile([128, N_ITEM], F32, tag="pt")
        nc.sync.dma_start(p[:], pt[bi * 128:(bi + 1) * 128, :])
        for ci, (c0, cn) in enumerate(CCH):
            tp = ptr.tile([128, 128], F32, tag="tr")
            nc.tensor.transpose(tp[:cn, :128], p[:, c0:c0 + cn], ident[:, :])
            nc.vector.tensor_copy(x_sb[ci][:cn, bi * 128:(bi + 1) * 128],
                                  tp[:cn, :128])

    # presum (row sums of P) as row vector via ones-matmul on X
    psum_ps = pbig.tile([1, BS], F32, tag="big")
    for ci, (c0, cn) in enumerate(CCH):
        for ns, nn in _nsl(BS):
            nc.tensor.matmul(psum_ps[:, ns:ns + nn], ones_col[:cn, :],
                             x_sb[ci][:cn, ns:ns + nn],
                             start=(ci == 0), stop=(ci == len(CCH) - 1))
    rpre = sb.tile([1, BS], F32, tag="rpre")
    nc.vector.tensor_copy(rpre[:], psum_ps[:])
    nc.vector.reciprocal(rpre[:], rpre[:])
    repp_ps = pbig.tile([D, BS], F32, tag="big")
    for ns, nn in _nsl(BS):
        nc.tensor.matmul(repp_ps[:, ns:ns + nn], ones_row[:, :],
                         rpre[:, ns:ns + nn], start=True, stop=True)
    repp = big.tile([D, BS], F32, tag="repp_zbn")
    nc.vector.tensor_copy(repp[:], repp_ps[:])

    # e_syndT = es^T @ X  -> yT = e_syndT / presum
    esy_ps = pbig.tile([D, BS], F32, tag="big")
    for ns, nn in _nsl(BS):
        for ci, (c0, cn) in enumerate(CCH):
            nc.tensor.matmul(esy_ps[:, ns:ns + nn], es_sb[ci][:cn, :D],
                             x_sb[ci][:cn, ns:ns + nn],
                             start=(ci == 0), stop=(ci == len(CCH) - 1))
    yt = big.tile([D, BS], F32, tag="yt_sq")
    nc.vector.tensor_mul(yt[:], esy_ps[:], repp[:])

    # zT = mlp_W @ yT + mlp_b
    zp_ps = pbig.tile([D, BS], F32, tag="big")
    for ns, nn in _nsl(BS):
        nc.tensor.matmul(zp_ps[:, ns:ns + nn], mwt_sb[:, :],
                         yt[:, ns:ns + nn], start=True, stop=True)
    zt = big.tile([D, BS], F32, tag="zt")
    nc.scalar.activation(zt[:], zp_ps[:], AF.Identity, bias=mb)

    # ---- BN stats + all-reduce ----
    stats = sb.tile([D, 2], F32, tag="stats")
    nc.vector.tensor_reduce(stats[:, 0:1], zt[:], axis=AX.X, op=ALU.add)
    sqz = big.tile([D, BS], F32, tag="yt_sq")
    nc.scalar.activation(sqz[:], zt[:], AF.Square, accum_out=stats[:, 1:2])
    st_in = dram.tile([D, 2], F32, tag="cc_in")
    st_out = dram.tile([D, 2], F32, tag="cc_out")
    nc.gpsimd.dma_start(st_in[:], stats[:])
    nc.gpsimd.collective_compute(
        "AllReduce", mybir.AluOpType.add,
        replica_groups=[list(range(NCORES))],
        ins=[st_in.opt()], outs=[st_out.opt()])
    ast = sb.tile([D, 2], F32, tag="ast")
    nc.gpsimd.dma_start(ast[:], st_out[:])

    mu = sb.tile([D, 4], F32, tag="mu")  # cols: mu, musq, var, scale
    nc.scalar.mul(mu[:, 0:1], ast[:, 0:1], 1.0 / B)
    nc.scalar.activation(mu[:, 1:2], mu[:, 0:1], AF.Square)
    nc.scalar.mul(mu[:, 2:3], ast[:, 1:2], 1.0 / B)
    nc.vector.tensor_sub(mu[:, 2:3], mu[:, 2:3], mu[:, 1:2])
    epst = sb.tile([D, 1], F32, tag="epst")
    nc.vector.memset(epst[:], BN_EPS)
    nc.scalar.activation(mu[:, 3:4], mu[:, 2:3], AF.Sqrt, bias=epst[:, 0:1])
    nc.vector.reciprocal(mu[:, 3:4], mu[:, 3:4])
    bnsc = sb.tile([D, 2], F32, tag="bnsc")  # scale, shift
    nc.vector.tensor_mul(bnsc[:, 0:1], gam, mu[:, 3:4])
    nc.vector.tensor_mul(bnsc[:, 1:2], mu[:, 0:1], bnsc[:, 0:1])
    nc.vector.tensor_sub(bnsc[:, 1:2], bet, bnsc[:, 1:2])

    zbn = big.tile([D, BS], F32, tag="repp_zbn")
    nc.scalar.activation(zbn[:], zt[:], AF.Relu,
                         bias=bnsc[:, 1:2], scale=bnsc[:, 0:1])

    # ---- out = z @ eh^T : per 128-row tile, lhsT = zbn[:, tile] ----
    for bi in range(BS // 128):
        o_ps = pools["pout"].tile([128, N_USER], F32, tag="ops")
        for ns, nn in _nsl(N_USER):
            nc.tensor.matmul(o_ps[:, ns:ns + nn],
                             zbn[:, bi * 128:(bi + 1) * 128],
                             eht[:, ns:ns + nn], start=True, stop=True)
        o_sb = outp.tile([128, N_USER], F32, tag="osb")
        nc.vector.tensor_copy(o_sb[:], o_ps[:])
        nc.sync.dma_start(out[bi * 128:(bi + 1) * 128, :], o_sb[:])


_NC_CACHE = {}


def _get_nc():
    if "nc" not in _NC_CACHE:
        _NC_CACHE["nc"] = _build()
    return _NC_CACHE["nc"]


def _prep(inputs):
    x_SH = np.asarray(inputs["x_SH"])
    ei = np.asarray(inputs["edge_index_SH"])
    presc = np.asarray(inputs["prescription"], dtype=np.float32)
    SH_emb = np.asarray(inputs["SH_emb"], dtype=np.float32)
    W1 = np.asarray(inputs["W1"], dtype=np.float32)
    b1 = np.asarray(inputs["b1"], dtype=np.float32)
    W2 = np.asarray(inputs["W2"], dtype=np.float32)
    b2 = np.asarray(inputs["b2"], dtype=np.float32)
    mlp_W = np.asarray(inputs["mlp_W"], dtype=np.float32)
    mlp_b = np.asarray(inputs["mlp_b"], dtype=np.float32)
    gam = np.asarray(inputs["bn_gamma"], dtype=np.float32)
    bet = np.asarray(inputs["bn_beta"], dtype=np.float32)

    x1 = SH_emb[np.asarray(x_SH, dtype=np.int64)]
    src = np.asarray(ei[0], dtype=np.int64)
    dst = np.asarray(ei[1], dtype=np.int64)
    stm = np.bincount(src * N_SH + dst, minlength=N_SH * N_SH)
    stm = stm.reshape(N_SH, N_SH).astype(np.float32)  # S^T[s,d]

    vecs = np.stack([b1, b2, mlp_b, gam, bet], axis=1).astype(np.float32)
    shared = {
        "st": np.ascontiguousarray(stm),
        "emb": np.ascontiguousarray(x1),
        "w1t": np.ascontiguousarray(W1.T),
        "w2t": np.ascontiguousarray(W2.T),
        "mwt": np.ascontiguousarray(mlp_W.T),
        "vecs": vecs,
    }
    in_maps = []
    for c in range(NCORES):
        m = dict(shared)
        m["pt"] = np.ascontiguousarray(presc[c * BS:(c + 1) * BS])
        in_maps.append(m)
    return in_maps


def kernel(**inputs):
    in_maps = _prep(inputs)
    nc = _get_nc()
    res = run_bass_kernel_spmd(nc, in_maps, list(range(NCORES)))
    outs = [res.results[c]["out"] for c in range(NCORES)]
    return np.concatenate(outs, axis=0).astype(np.float32)


def run_traced(inputs, tmpdir=None):
    """Profiled run: returns (output, exec_time_ns, results_obj)."""
    in_maps = _prep(inputs)
    nc = _get_nc()
    res = run_bass_kernel_spmd(nc, in_maps, list(range(NCORES)),
                               trace=True, tmpdir=tmpdir)
    outs = [res.results[c]["out"] for c in range(NCORES)]
    full = np.concatenate(outs, axis=0).astype(np.float32)
    return full, res.exec_time_ns, res


# revision 12
# speedup vs baseline: 358.2822x; 358.2822x over previous
"""Trainium2 Bass kernel for nn_KDHR (gnn_message_passing).

Math reduction: with S[d,s] = #edges (s->d) over N_SH=1195 nodes,
each GCN-mean layer is  h = tanh((S @ x @ W.T + cnt*b) / max(cnt,1)),
cnt = row sums of S.  So the 1M-edge message passing collapses to a
dense (1195,1195) count matrix (built once) + small dense matmuls.

Device layout: everything feature-major ("T layout", features on the
128-partition axis) so BatchNorm / bias / norms are per-partition ops.
Batch (16384) is sharded 2048 rows/core across 8 cores; BN statistics
are all-reduced (one tiny [64,2] collective).
"""

import os
import sys

for _p in ("/root/.axon_site", "/root/.axon_site/_ro/trn_rl_repo",
           "/root/.axon_site/_ro/pypackages", "/opt/trn_rl_repo", "/opt/pypackages"):
    if os.path.isdir(_p) and _p not in sys.path:
        sys.path.append(_p)

import numpy as np

import concourse.bass as bass
import concourse.mybir as mybir
import concourse.tile as tile
from concourse import bacc
from concourse.bass_utils import run_bass_kernel_spmd
from concourse.masks import make_identity

N_USER, N_ITEM, N_SH, D = 805, 390, 1195, 64
B, E, NCORES = 16384, 1048576, 8
BS = B // NCORES  # 2048 batch rows per core
BN_EPS = 1e-5
NORM_EPS = 1e-12
F32 = mybir.dt.float32
BF16 = mybir.dt.bfloat16

# contraction chunks over the node dim (1195 = 9*128 + 43)
KCH = [(k, min(128, N_SH - k)) for k in range(0, N_SH, 128)]
# chunks over the item dim (390 = 3*128 + 6)
CCH = [(c, min(128, N_ITEM - c)) for c in range(0, N_ITEM, 128)]


def _nsl(n, step=512):
    return [(s, min(step, n - s)) for s in range(0, n, step)]


def _build(collective=True):
    nc = bacc.Bacc("TRN2", target_bir_lowering=False, debug=False,
                   num_devices=NCORES)

    pt = nc.declare_dram_parameter("pt", [BS, N_ITEM], F32, isOutput=False).ap()
    st = nc.declare_dram_parameter("st", [N_SH, N_SH], BF16, isOutput=False).ap()
    emb = nc.declare_dram_parameter("emb", [N_SH, D], F32, isOutput=False).ap()
    w1t = nc.declare_dram_parameter("w1t", [D, D], F32, isOutput=False).ap()
    w2t = nc.declare_dram_parameter("w2t", [D, D], F32, isOutput=False).ap()
    mwt = nc.declare_dram_parameter("mwt", [D, D], F32, isOutput=False).ap()
    vecs = nc.declare_dram_parameter("vecs", [D, 5], F32, isOutput=False).ap()
    out = nc.declare_dram_parameter("out", [BS, N_USER], F32, isOutput=True).ap()

    from contextlib import ExitStack
    with tile.TileContext(nc) as tc, ExitStack() as ctx:
        pools = {
            "cst": ctx.enter_context(tc.tile_pool(name="cst", bufs=1)),
            "big": ctx.enter_context(tc.tile_pool(name="big", bufs=1)),
            "sb": ctx.enter_context(tc.tile_pool(name="sb", bufs=1)),
            "ptp": ctx.enter_context(tc.tile_pool(name="ptp", bufs=3)),
            "xp": ctx.enter_context(tc.tile_pool(name="xp", bufs=1)),
            "outp": ctx.enter_context(tc.tile_pool(name="outp", bufs=3)),
            "ptr": ctx.enter_context(tc.tile_pool(name="ptr", bufs=2, space="PSUM")),
            "pbig": ctx.enter_context(tc.tile_pool(name="pbig", bufs=1, space="PSUM")),
            "pout": ctx.enter_context(tc.tile_pool(name="pout", bufs=1, space="PSUM")),
            "dram": ctx.enter_context(tc.tile_pool(name="dram", bufs=1, space="DRAM")),
        }
        _body(nc, tc, pools, pt, st, emb, w1t, w2t, mwt, vecs, out, collective)

    nc.compile()
    return nc


def _body(nc, tc, pools, pt, st, emb, w1t, w2t, mwt, vecs, out, collective=True):
    AF = mybir.ActivationFunctionType
    ALU = mybir.AluOpType
    AX = mybir.AxisListType
    cst, big, sb = pools["cst"], pools["big"], pools["sb"]
    ptp, xp, outp = pools["ptp"], pools["xp"], pools["outp"]
    ptr, pbig, dram = pools["ptr"], pools["pbig"], pools["dram"]

    # ---- constants ----
    ident = cst.tile([128, 128], F32, tag="ident")
    make_identity(nc, ident[:])
    ones_col = cst.tile([128, 1], F32, tag="ones_col")   # column of ones
    nc.vector.memset(ones_col[:], 1.0)
    ones_row = cst.tile([1, D], F32, tag="ones_row")     # 1 x 64 of ones
    nc.vector.memset(ones_row[:], 1.0)
    ones_colb = cst.tile([128, 1], BF16, tag="ones_colb")
    nc.vector.memset(ones_colb[:], 1.0)

    w1t_sb = cst.tile([D, D], F32, tag="w1t")
    nc.sync.dma_start(w1t_sb[:], w1t[:, :])
    w2t_sb = cst.tile([D, D], F32, tag="w2t")
    nc.sync.dma_start(w2t_sb[:], w2t[:, :])
    mwt_sb = cst.tile([D, D], F32, tag="mwt")
    nc.sync.dma_start(mwt_sb[:], mwt[:, :])
    vec_sb = cst.tile([D, 5], F32, tag="vecs")           # b1,b2,mb,gam,bet
    nc.sync.dma_start(vec_sb[:], vecs[:, :])
    b1 = vec_sb[:, 0:1]
    b2 = vec_sb[:, 1:2]
    mb = vec_sb[:, 2:3]
    gam = vec_sb[:, 3:4]
    bet = vec_sb[:, 4:5]

    # ---- load S^T (10 chunks of [<=128, 1195]) and emb chunks ----
    st_sb, x1_sb, x1b_sb = [], [], []
    for i, (k0, kn) in enumerate(KCH):
        t = sb.tile([128, N_SH], BF16, tag=f"st{i}")
        nc.sync.dma_start(t[:kn, :], st[k0:k0 + kn, :])
        st_sb.append(t)
        x = sb.tile([128, D], F32, tag=f"x1{i}")
        nc.sync.dma_start(x[:kn, :], emb[k0:k0 + kn, :])
        x1_sb.append(x)
        xb = sb.tile([128, D], BF16, tag=f"x1b{i}")
        nc.vector.tensor_copy(xb[:kn, :], x[:kn, :])
        xr = sb.tile([128, D], F32, tag="x1r")
        nc.vector.tensor_sub(xr[:kn, :], x[:kn, :], xb[:kn, :])
        xl = sb.tile([128, D], BF16, tag=f"x1l{i}")
        nc.vector.tensor_copy(xl[:kn, :], xr[:kn, :])
        x1b_sb.append((xb, xl))

    # ---- cnt = column sums of S^T -> recm[64,1195] = bcast 1/max(cnt,1) ----
    cnt_ps = pbig.tile([1, N_SH], F32, tag="big")
    for i, (k0, kn) in enumerate(KCH):
        for ns, nn in _nsl(N_SH):
            nc.tensor.matmul(cnt_ps[:, ns:ns + nn], ones_colb[:kn, :],
                             st_sb[i][:kn, ns:ns + nn],
                             start=(i == 0), stop=(i == len(KCH) - 1))
    rec1 = sb.tile([1, N_SH], F32, tag="rec1")
    nc.vector.tensor_scalar_max(rec1[:], cnt_ps[:], 1.0)
    nc.vector.reciprocal(rec1[:], rec1[:])
    rep_ps = pbig.tile([D, N_SH], F32, tag="big")
    for ns, nn in _nsl(N_SH):
        nc.tensor.matmul(rep_ps[:, ns:ns + nn], ones_row[:, :],
                         rec1[:, ns:ns + nn], start=True, stop=True)
    recm = big.tile([D, N_SH], F32, tag="recm")
    nc.vector.tensor_copy(recm[:], rep_ps[:])

    # ---- x1T via PE transpose ----
    x1t = big.tile([D, N_SH], F32, tag="x1t")
    for i, (k0, kn) in enumerate(KCH):
        tp = ptr.tile([128, 128], F32, tag="tr")
        nc.tensor.transpose(tp[:D, :kn], x1_sb[i][:kn, :D], ident[:kn, :kn])
        nc.vector.tensor_copy(x1t[:, k0:k0 + kn], tp[:D, :kn])

    # ---- layer 1: AT = x1^T @ S^T ; h1T = tanh(AT*W1^T-ish scaled) ----
    at_ps = pbig.tile([D, N_SH], F32, tag="big")
    for p in range(2):
        for i, (k0, kn) in enumerate(KCH):
            for ns, nn in _nsl(N_SH):
                nc.tensor.matmul(at_ps[:, ns:ns + nn], x1b_sb[i][p][:kn, :D],
                                 st_sb[i][:kn, ns:ns + nn],
                                 start=(p == 0 and i == 0),
                                 stop=(p == 1 and i == len(KCH) - 1))
    at_sb = big.tile([D, N_SH], F32, tag="ab")
    nc.vector.tensor_copy(at_sb[:], at_ps[:])
    h1p_ps = pbig.tile([D, N_SH], F32, tag="big")
    for ns, nn in _nsl(N_SH):
        nc.tensor.matmul(h1p_ps[:, ns:ns + nn], w1t_sb[:, :],
                         at_sb[:, ns:ns + nn], start=True, stop=True)
    h1t = big.tile([D, N_SH], F32, tag="h1t")
    nc.vector.tensor_mul(h1t[:], h1p_ps[:], recm[:])
    nc.scalar.activation(h1t[:], h1t[:], AF.Tanh, bias=b1)

    # transpose h1T -> h1 natural (lhsT for layer 2)
    h1_sb = []
    for i, (k0, kn) in enumerate(KCH):
        tp = ptr.tile([128, 128], F32, tag="tr")
        nc.tensor.transpose(tp[:kn, :D], h1t[:, k0:k0 + kn], ident[:D, :D])
        h = sb.tile([128, D], BF16, tag=f"h1_{i}")
        nc.vector.tensor_copy(h[:kn, :], tp[:kn, :D])
        hr = sb.tile([128, D], F32, tag="h1r")
        nc.vector.tensor_sub(hr[:kn, :], tp[:kn, :D], h[:kn, :])
        hl = sb.tile([128, D], BF16, tag=f"h1l_{i}")
        nc.vector.tensor_copy(hl[:kn, :], hr[:kn, :])
        h1_sb.append((h, hl))

    # ---- layer 2 ----
    bt_ps = pbig.tile([D, N_SH], F32, tag="big")
    for p in range(2):
        for i, (k0, kn) in enumerate(KCH):
            for ns, nn in _nsl(N_SH):
                nc.tensor.matmul(bt_ps[:, ns:ns + nn], h1_sb[i][p][:kn, :D],
                                 st_sb[i][:kn, ns:ns + nn],
                                 start=(p == 0 and i == 0),
                                 stop=(p == 1 and i == len(KCH) - 1))
    bt_sb = big.tile([D, N_SH], F32, tag="ab")
    nc.vector.tensor_copy(bt_sb[:], bt_ps[:])
    h2p_ps = pbig.tile([D, N_SH], F32, tag="big")
    for ns, nn in _nsl(N_SH):
        nc.tensor.matmul(h2p_ps[:, ns:ns + nn], w2t_sb[:, :],
                         bt_sb[:, ns:ns + nn], start=True, stop=True)
    h2t = big.tile([D, N_SH], F32, tag="h2t")
    nc.vector.tensor_mul(h2t[:], h2p_ps[:], recm[:])
    nc.scalar.activation(h2t[:], h2t[:], AF.Tanh, bias=b2)

    # ---- norms -> esT/ehT ----
    # row norms of x1 (per node) as a row vector, via ones-matmul on x1t^2
    sqx = big.tile([D, N_SH], F32, tag="sq")
    nc.scalar.activation(sqx[:], x1t[:], AF.Square)
    rn_ps = pbig.tile([1, N_SH], F32, tag="big")
    for ns, nn in _nsl(N_SH):
        nc.tensor.matmul(rn_ps[:, ns:ns + nn], ones_col[:D, :],
                         sqx[:, ns:ns + nn], start=True, stop=True)
    rn = sb.tile([1, N_SH], F32, tag="rn")
    nc.scalar.activation(rn[:], rn_ps[:], AF.Sqrt)
    nc.vector.tensor_scalar_max(rn[:], rn[:], NORM_EPS)
    nc.vector.reciprocal(rn[:], rn[:])
    repn_ps = pbig.tile([D, N_SH], F32, tag="big")
    for ns, nn in _nsl(N_SH):
        nc.tensor.matmul(repn_ps[:, ns:ns + nn], ones_row[:, :],
                         rn[:, ns:ns + nn], start=True, stop=True)
    repn = big.tile([D, N_SH], F32, tag="repn")
    nc.vector.tensor_copy(repn[:], repn_ps[:])

    # column norms of h2 (per feature), separately for user/item slices
    hsq = big.tile([D, N_SH], F32, tag="sq")
    nc.scalar.activation(hsq[:], h2t[:], AF.Square)
    rcu = sb.tile([D, 2], F32, tag="rcu")
    nc.vector.tensor_reduce(rcu[:, 0:1], hsq[:, 0:N_USER], axis=AX.X, op=ALU.add)
    nc.vector.tensor_reduce(rcu[:, 1:2], hsq[:, N_USER:N_SH], axis=AX.X, op=ALU.add)
    nc.scalar.activation(rcu[:], rcu[:], AF.Sqrt)
    nc.vector.tensor_scalar_max(rcu[:], rcu[:], NORM_EPS)
    nc.vector.reciprocal(rcu[:], rcu[:])

    # ehT[64,805] / esT[64,390]
    eht = big.tile([D, N_USER], F32, tag="eht")
    nc.vector.tensor_mul(eht[:], x1t[:, 0:N_USER], repn[:, 0:N_USER])
    tmpu = big.tile([D, N_USER], F32, tag="tmp")
    nc.vector.tensor_scalar_mul(tmpu[:], h2t[:, 0:N_USER], rcu[:, 0:1])
    nc.vector.tensor_add(eht[:], eht[:], tmpu[:])
    est = big.tile([D, N_ITEM], F32, tag="est")
    nc.vector.tensor_mul(est[:], x1t[:, N_USER:N_SH], repn[:, N_USER:N_SH])
    tmpi = big.tile([D, N_ITEM], F32, tag="tmp")
    nc.vector.tensor_scalar_mul(tmpi[:], h2t[:, N_USER:N_SH], rcu[:, 1:2])
    nc.vector.tensor_add(est[:], est[:], tmpi[:])

    # es natural [390,64] (lhsT for e_synd), via PE transpose
    es_sb = []
    for i, (c0, cn) in enumerate(CCH):
        tp = ptr.tile([128, 128], F32, tag="tr")
        nc.tensor.transpose(tp[:cn, :D], est[:, c0:c0 + cn], ident[:D, :D])
        e = sb.tile([128, D], F32, tag=f"es{i}")
        nc.vector.tensor_copy(e[:cn, :], tp[:cn, :D])
        es_sb.append(e)

    # ---- batch stage: X = P^T in SBUF via PE transposes ----
    x_sb = [xp.tile([128, BS], F32, tag=f"X{i}", name=f"X{i}")
            for i in range(len(CCH))]
    for bi in range(BS // 128):
        p = ptp.tile([128, N_ITEM], F32, tag="pt")
        nc.sync.dma_start(p[:], pt[bi * 128:(bi + 1) * 128, :])
        for ci, (c0, cn) in enumerate(CCH):
            tp = ptr.tile([128, 128], F32, tag="tr")
            nc.tensor.transpose(tp[:cn, :128], p[:, c0:c0 + cn], ident[:, :])
            nc.vector.tensor_copy(x_sb[ci][:cn, bi * 128:(bi + 1) * 128],
                                  tp[:cn, :128])

    # presum (row sums of P) as row vector via ones-matmul on X
    psum_ps = pbig.tile([1, BS], F32, tag="big")
    for ci, (c0, cn) in enumerate(CCH):
        for ns, nn in _nsl(BS):
            nc.tensor.matmul(psum_ps[:, ns:ns + nn], ones_col[:cn, :],
                             x_sb[ci][:cn, ns:ns + nn],
                             start=(ci == 0), stop=(ci == len(CCH) - 1))
    rpre = sb.tile([1, BS], F32, tag="rpre")
    nc.vector.tensor_copy(rpre[:], psum_ps[:])
    nc.vector.reciprocal(rpre[:], rpre[:])
    repp_ps = pbig.tile([D, BS], F32, tag="big")
    for ns, nn in _nsl(BS):
        nc.tensor.matmul(repp_ps[:, ns:ns + nn], ones_row[:, :],
                         rpre[:, ns:ns + nn], start=True, stop=True)
    repp = big.tile([D, BS], F32, tag="repp_zbn")
    nc.vector.tensor_copy(repp[:], repp_ps[:])

    # e_syndT = es^T @ X  -> yT = e_syndT / presum
    esy_ps = pbig.tile([D, BS], F32, tag="big")
    for ns, nn in _nsl(BS):
        for ci, (c0, cn) in enumerate(CCH):
            nc.tensor.matmul(esy_ps[:, ns:ns + nn], es_sb[ci][:cn, :D],
                             x_sb[ci][:cn, ns:ns + nn],
                             start=(ci == 0), stop=(ci == len(CCH) - 1))
    yt = big.tile([D, BS], F32, tag="yt_sq")
    nc.vector.tensor_mul(yt[:], esy_ps[:], repp[:])

    # zT = mlp_W @ yT + mlp_b
    zp_ps = pbig.tile([D, BS], F32, tag="big")
    for ns, nn in _nsl(BS):
        nc.tensor.matmul(zp_ps[:, ns:ns + nn], mwt_sb[:, :],
                         yt[:, ns:ns + nn], start=True, stop=True)
    zt = big.tile([D, BS], F32, tag="zt")
    nc.scalar.activation(zt[:], zp_ps[:], AF.Identity, bias=mb)

    # ---- BN stats + all-reduce ----
    stats = sb.tile([D, 2], F32, tag="stats")
    nc.vector.tensor_reduce(stats[:, 0:1], zt[:], axis=AX.X, op=ALU.add)
    sqz = big.tile([D, BS], F32, tag="yt_sq")
    nc.scalar.activation(sqz[:], zt[:], AF.Square, accum_out=stats[:, 1:2])
    st_in = dram.tile([D, 2], F32, tag="cc_in")
    st_out = dram.tile([D, 2], F32, tag="cc_out")
    nc.gpsimd.dma_start(st_in[:], stats[:])
    if collective:
        nc.gpsimd.collective_compute(
            "AllReduce", mybir.AluOpType.add,
            replica_groups=[list(range(NCORES))],
            ins=[st_in.opt()], outs=[st_out.opt()])
    else:
        nc.gpsimd.dma_start(st_out[:], st_in[:])
    ast = sb.tile([D, 2], F32, tag="ast")
    nc.gpsimd.dma_start(ast[:], st_out[:])

    mu = sb.tile([D, 4], F32, tag="mu")  # cols: mu, musq, var, scale
    nc.scalar.mul(mu[:, 0:1], ast[:, 0:1], 1.0 / B)
    nc.scalar.activation(mu[:, 1:2], mu[:, 0:1], AF.Square)
    nc.scalar.mul(mu[:, 2:3], ast[:, 1:2], 1.0 / B)
    nc.vector.tensor_sub(mu[:, 2:3], mu[:, 2:3], mu[:, 1:2])
    epst = sb.tile([D, 1], F32, tag="epst")
    nc.vector.memset(epst[:], BN_EPS)
    nc.scalar.activation(mu[:, 3:4], mu[:, 2:3], AF.Sqrt, bias=epst[:, 0:1])
    nc.vector.reciprocal(mu[:, 3:4], mu[:, 3:4])
    bnsc = sb.tile([D, 2], F32, tag="bnsc")  # scale, shift
    nc.vector.tensor_mul(bnsc[:, 0:1], gam, mu[:, 3:4])
    nc.vector.tensor_mul(bnsc[:, 1:2], mu[:, 0:1], bnsc[:, 0:1])
    nc.vector.tensor_sub(bnsc[:, 1:2], bet, bnsc[:, 1:2])

    zbn = big.tile([D, BS], F32, tag="repp_zbn")
    nc.scalar.activation(zbn[:], zt[:], AF.Relu,
                         bias=bnsc[:, 1:2], scale=bnsc[:, 0:1])

    # ---- out = z @ eh^T : per 128-row tile, lhsT = zbn[:, tile] ----
    for bi in range(BS // 128):
        o_ps = pools["pout"].tile([128, N_USER], F32, tag="ops")
        for ns, nn in _nsl(N_USER):
            nc.tensor.matmul(o_ps[:, ns:ns + nn],
                             zbn[:, bi * 128:(bi + 1) * 128],
                             eht[:, ns:ns + nn], start=True, stop=True)
        o_sb = outp.tile([128, N_USER], F32, tag="osb")
        nc.vector.tensor_copy(o_sb[:], o_ps[:])
        nc.sync.dma_start(out[bi * 128:(bi + 1) * 128, :], o_sb[:])


_NC_CACHE = {}


def _get_nc():
    if "nc" not in _NC_CACHE:
        _NC_CACHE["nc"] = _build()
    return _NC_CACHE["nc"]


def _prep(inputs):
    x_SH = np.asarray(inputs["x_SH"])
    ei = np.asarray(inputs["edge_index_SH"])
    presc = np.asarray(inputs["prescription"], dtype=np.float32)
    SH_emb = np.asarray(inputs["SH_emb"], dtype=np.float32)
    W1 = np.asarray(inputs["W1"], dtype=np.float32)
    b1 = np.asarray(inputs["b1"], dtype=np.float32)
    W2 = np.asarray(inputs["W2"], dtype=np.float32)
    b2 = np.asarray(inputs["b2"], dtype=np.float32)
    mlp_W = np.asarray(inputs["mlp_W"], dtype=np.float32)
    mlp_b = np.asarray(inputs["mlp_b"], dtype=np.float32)
    gam = np.asarray(inputs["bn_gamma"], dtype=np.float32)
    bet = np.asarray(inputs["bn_beta"], dtype=np.float32)

    x1 = SH_emb[np.asarray(x_SH, dtype=np.int64)]
    src = np.asarray(ei[0], dtype=np.int64)
    dst = np.asarray(ei[1], dtype=np.int64)
    stm = np.bincount(src * N_SH + dst, minlength=N_SH * N_SH)
    import ml_dtypes
    stm = stm.reshape(N_SH, N_SH).astype(ml_dtypes.bfloat16)  # S^T[s,d]

    vecs = np.stack([b1, b2, mlp_b, gam, bet], axis=1).astype(np.float32)
    shared = {
        "st": np.ascontiguousarray(stm),
        "emb": np.ascontiguousarray(x1),
        "w1t": np.ascontiguousarray(W1.T),
        "w2t": np.ascontiguousarray(W2.T),
        "mwt": np.ascontiguousarray(mlp_W.T),
        "vecs": vecs,
    }
    in_maps = []
    for c in range(NCORES):
        m = dict(shared)
        m["pt"] = np.ascontiguousarray(presc[c * BS:(c + 1) * BS])
        in_maps.append(m)
    return in_maps


def kernel(**inputs):
    in_maps = _prep(inputs)
    nc = _get_nc()
    res = run_bass_kernel_spmd(nc, in_maps, list(range(NCORES)))
    outs = [res.results[c]["out"] for c in range(NCORES)]
    return np.concatenate(outs, axis=0).astype(np.float32)


def run_traced(inputs, tmpdir=None):
    """Profiled run: returns (output, exec_time_ns, results_obj)."""
    in_maps = _prep(inputs)
    nc = _get_nc()
    res = run_bass_kernel_spmd(nc, in_maps, list(range(NCORES)),
                               trace=True, tmpdir=tmpdir)
    outs = [res.results[c]["out"] for c in range(NCORES)]
    full = np.concatenate(outs, axis=0).astype(np.float32)
    return full, res.exec_time_ns, res
